# revision 1
# baseline (speedup 1.0000x reference)
"""EfficientAttention (linear attention) Trainium2 Bass kernel.

Computes, per batch b:
    q_n = softmax(q[b], axis=-1)        # over feature dim D=64
    k_n = softmax(k[b], axis=-1)
    ctx = k_n^T @ v[b]                  # [D, D]
    out[b] = q_n @ ctx                  # [N, D]

Sharding: batch dim (32) split across 8 cores, 4 batches per core.

Design notes (per core):
- DMA: 1 MB loads/stores ([128 partitions, 8 KB contiguous per partition];
  rows interleaved so partition p holds rows n0+32p .. n0+32p+31).
- fp32 matmuls on the PE run as two half-speed passes (fp32_mode=LOW_HIGH);
  fp16 runs single-pass with fast-weight-load. All matmul inputs are cast
  to fp16 at zero cost: ACT exp writes fp16, the DVE normalize writes fp16,
  and v is cast f32->f16 in-flight by a SWDGE (gpsimd) DMA. PSUM
  accumulation stays fp32. Raw-q transposes stay fp32 (exp-first variants measured slower).
- K/V pass: exp(k) on ACT, row-sums + reciprocal + scale on DVE,
  PE accumulates ctx[64,64] over N.
- ctx epilogue: block-diagonal stacked ctxa [128, 130] fp16
  (rows 0:64 = [ctx | 1 | 0], rows 64:128 = [0 | ctx | 1]) so one K=128
  matmul computes two packed row-tiles (cols 0:65 and 65:130) with a
  single full row group. (Matmuls with alternating row groups writing one
  PSUM bank lock up the device - found by bisection.)
- Q pass: PE-transpose raw q pairs [128, 2x64] -> PSUM [128,128] (feature
  dim onto partitions, two row-tiles stacked), ACT exp PSUM->SBUF (fused
  evict + exp + fp16 cast), one matmul per pair against ctxa -> [128,130]
  (col 64/129 = row-sums via the ones columns), DVE reciprocal + multiply
  -> natural-layout fp32 output.
- Batch b's q-pass is interleaved with batch b+1's k/v-pass to keep the
  PE dense (HAM stays un-throttled) and the DMA queues evenly loaded.
"""

import numpy as np

import concourse.bass as bass
import concourse.mybir as mybir
import concourse.tile as tile
from concourse import bacc
from concourse.bass_utils import run_bass_kernel_spmd

B, N, D = 32, 16384, 64
NCORES = 8
BPC = B // NCORES  # batches per core
LOAD = 4096  # rows per DMA (1 MB fp32)
LT = LOAD // 128  # row-tile slots per load (32)
NBLK = N // LOAD  # load blocks per batch (4)
F32 = mybir.dt.float32
F16 = mybir.dt.float16
EXP = mybir.ActivationFunctionType.Exp


def build_bass():
    nc = bacc.Bacc("TRN2", target_bir_lowering=False, debug=False)
    q = nc.dram_tensor("q", [BPC, N, D], F32, kind="ExternalInput").ap()
    k = nc.dram_tensor("k", [BPC, N, D], F32, kind="ExternalInput").ap()
    v = nc.dram_tensor("v", [BPC, N, D], F32, kind="ExternalInput").ap()
    o = nc.dram_tensor("o", [BPC, N, D], F32, kind="ExternalOutput").ap()

    def blk(t, b, n0):
        return t[b, n0 : n0 + LOAD, :].rearrange("(p t) d -> p t d", p=128)

    with tile.TileContext(nc) as tc:
        with (
            tc.tile_pool(name="consts", bufs=1) as consts,
            tc.tile_pool(name="io", bufs=2) as io,
            tc.tile_pool(name="work", bufs=3) as work,
            tc.tile_pool(name="ctxp", bufs=2) as ctxp,
            tc.tile_pool(name="ps_t", bufs=2, space="PSUM") as ps_t,
            tc.tile_pool(name="ps_o", bufs=4, space="PSUM") as ps_o,
            tc.tile_pool(name="ps_c", bufs=2, space="PSUM") as ps_c,
        ):
            from concourse.masks import make_identity

            ident = consts.tile([128, 128], F32)
            make_identity(nc, ident)

            ctx_ps = {}

            def emit_kv_block(b, i):
                n0 = i * LOAD
                k_sb = io.tile([128, LT, 64], F32, tag="k_sb", bufs=4)
                v_sb = io.tile([128, LT, 64], F16, tag="v_sb", bufs=4)
                nc.sync.dma_start(out=k_sb, in_=blk(k, b, n0))
                # SWDGE DMA casts f32 -> f16 in flight
                nc.gpsimd.dma_start(out=v_sb, in_=blk(v, b, n0))
                ek = work.tile([128, LT, 64], F32, tag="ek")
                nc.scalar.activation(ek, k_sb, EXP)
                ks = work.tile([128, LT, 1], F32, tag="ks")
                nc.vector.reduce_sum(out=ks, in_=ek, axis=mybir.AxisListType.X)
                ksr = work.tile([128, LT, 1], F32, tag="ksr")
                nc.vector.reciprocal(ksr, ks)
                ekn = work.tile([128, LT, 64], F16, tag="ekn", bufs=4)
                nc.gpsimd.tensor_mul(ekn, ek, ksr[:].to_broadcast((128, LT, 64)))
                for t in range(LT):
                    nc.tensor.matmul(
                        ctx_ps[b],
                        ekn[:, t, :],
                        v_sb[:, t, :],
                        start=(i == 0 and t == 0),
                        stop=(i == NBLK - 1 and t == LT - 1),
                    )

            def emit_ctx_epilogue(b):
                ctxa = ctxp.tile([128, 130], F16, tag="ctxa")
                nc.vector.memset(ctxa, 0.0)
                nc.vector.tensor_copy(ctxa[0:64, 0:64], ctx_ps[b])
                nc.vector.memset(ctxa[0:64, 64:65], 1.0)
                nc.scalar.dma_start(out=ctxa[64:128, 65:130], in_=ctxa[0:64, 0:65])
                return ctxa

            def load_q_block(b, i):
                q_sb = io.tile([128, LT, 64], F32, tag="q_sb", bufs=4, name="q_sb")
                nc.sync.dma_start(out=q_sb, in_=blk(q, b, i * LOAD))
                return q_sb

            def emit_q_block(b, i, ctxa, q_sb=None, split_store=False):
                n0 = i * LOAD
                if q_sb is None:
                    q_sb = load_q_block(b, i)
                out_sb = io.tile([128, LT, 64], F32, tag="out_sb", bufs=3)
                for c in range(LT // 8):  # 1024-row compute chunks
                    tp_ps = ps_t.tile([128, 4, 128], F32, tag="tp_ps")
                    for u in range(4):
                        s0 = 8 * c + 2 * u
                        nc.tensor.transpose(
                            tp_ps[:, u, :],
                            q_sb[:, s0 : s0 + 2, :].rearrange("p t d -> p (t d)"),
                            ident,
                        )
                    eqT = work.tile([128, 4, 128], F16, tag="eqT", bufs=8)
                    nc.scalar.activation(eqT, tp_ps, EXP)
                    for g in range(2):
                        o_ps = ps_o.tile([128, 2, 132], F32, tag="o_ps")
                        for s in range(2):
                            nc.tensor.matmul(
                                o_ps[:, s, 0:130],
                                eqT[:, 2 * g + s, :],
                                ctxa,
                                start=True,
                                stop=True,
                            )
                        opb = o_ps[:]
                        pdim = opb.ap[0]
                        sstep = opb.ap[1][0]  # slot stride (132)
                        cstep = opb.ap[2][0]  # col stride (1)
                        r_sb = work.tile([128, 2, 2, 1], F32, tag="r_sb")
                        rs_ap = bass.AP(
                            tensor=opb.tensor,
                            offset=opb.offset + 64 * cstep,
                            ap=[pdim, [sstep, 2], [65 * cstep, 2], [cstep, 1]],
                        )
                        nc.vector.reciprocal(r_sb, rs_ap)
                        vals_ap = bass.AP(
                            tensor=opb.tensor,
                            offset=opb.offset,
                            ap=[pdim, [sstep, 2], [65 * cstep, 2], [cstep, 64]],
                        )
                        t0 = 8 * c + 4 * g
                        out_view = out_sb[:, t0 : t0 + 4, :].rearrange(
                            "p (s t) d -> p s t d", s=2
                        )
                        nc.vector.tensor_mul(
                            out_view,
                            vals_ap,
                            r_sb[:].to_broadcast((128, 2, 2, 64)),
                        )
                    if split_store:
                        nc.scalar.dma_start(
                            out=blk(o, b, n0)[:, 8 * c : 8 * c + 8, :],
                            in_=out_sb[:, 8 * c : 8 * c + 8, :],
                        )
                if not split_store:
                    nc.scalar.dma_start(out=blk(o, b, n0), in_=out_sb)

            # software-pipelined schedule: q-pass(b) interleaved with kv(b+1)
            ctx_ps[0] = ps_c.tile([64, 64], F32, tag="ctx_ps", name="ctx_ps")
            q_pre = [load_q_block(0, 0), load_q_block(0, 1)]
            for i in range(NBLK):
                emit_kv_block(0, i)
            ctxa = emit_ctx_epilogue(0)
            for b in range(BPC):
                if b + 1 < BPC:
                    ctx_ps[b + 1] = ps_c.tile([64, 64], F32, tag="ctx_ps", name="ctx_ps")
                nxt = None
                for i in range(NBLK):
                    # kv(b+1) first so its ctx completes before q(b) drains;
                    # epilogue right after the last kv block
                    if b + 1 < BPC:
                        emit_kv_block(b + 1, i)
                        if i == NBLK - 1:
                            nxt = emit_ctx_epilogue(b + 1)
                    last = b == BPC - 1 and i == NBLK - 1
                    emit_q_block(
                        b, i, ctxa,
                        q_sb=q_pre.pop(0) if (b == 0 and q_pre) else None,
                        split_store=last,
                    )
                if nxt is not None:
                    ctxa = nxt

    nc.compile()
    return nc


_NC_CACHE = None


def kernel(q: np.ndarray, k: np.ndarray, v: np.ndarray) -> np.ndarray:
    global _NC_CACHE
    if _NC_CACHE is None:
        _NC_CACHE = build_bass()
    nc = _NC_CACHE
    q = np.ascontiguousarray(np.asarray(q), dtype=np.float32)
    k = np.ascontiguousarray(np.asarray(k), dtype=np.float32)
    v = np.ascontiguousarray(np.asarray(v), dtype=np.float32)
    in_maps = [
        {
            "q": q[i * BPC : (i + 1) * BPC],
            "k": k[i * BPC : (i + 1) * BPC],
            "v": v[i * BPC : (i + 1) * BPC],
        }
        for i in range(NCORES)
    ]
    res = run_bass_kernel_spmd(nc, in_maps, core_ids=list(range(NCORES)))
    return np.concatenate([res.results[i]["o"] for i in range(NCORES)], axis=0)



# revision 2
# speedup vs baseline: 1.2583x; 1.2583x over previous
"""EfficientAttention (linear attention) Trainium2 Bass kernel.

Computes, per batch b:
    q_n = softmax(q[b], axis=-1)        # over feature dim D=64
    k_n = softmax(k[b], axis=-1)
    ctx = k_n^T @ v[b]                  # [D, D]
    out[b] = q_n @ ctx                  # [N, D]

Sharding: batch dim (32) split across 8 cores, 4 batches per core.

Design notes (per core):
- fp16 I/O: the host casts q/k/v to fp16 and the kernel stores fp16
  outputs (cast back to fp32 on host). Halves HBM traffic vs fp32 —
  this kernel is HBM-bound (32 MB @ ~358 GB/s/core ≈ 89 us floor).
  Input quantization adds ~1e-3 rel err, well inside the 2e-2 gate.
- DMA: 1 MB loads/stores ([128 partitions, 8 KB contiguous per
  partition]; partition p holds rows n0+64p .. n0+64p+63). k/q loads
  trigger via sync (SP HWDGE ring), v loads + o stores via scalar
  (ACT HWDGE ring) to balance the two rings.
- All matmuls run fp16 single-pass (fast weight load); PSUM
  accumulation fp32. No SWDGE casts needed anywhere.
- K/V pass: exp(k) on ACT (fp16 in/out), row-sums + reciprocal on DVE,
  normalize on Pool, PE accumulates ctx[64,64] over N.
- ctx epilogue: block-diagonal stacked ctxa [128, 130] fp16
  (rows 0:64 = [ctx | 1 | 0], rows 64:128 = [0 | ctx | 1]) so one K=128
  matmul computes two packed row-tiles (cols 0:65 and 65:130) with a
  single full row group. (Matmuls with alternating row groups writing
  one PSUM bank lock up the device - found by bisection.)
- Q pass: PE-transpose fp16 q pairs [128, 2x64] -> PSUM fp16 [128,128]
  (feature dim onto partitions, two row-tiles stacked), ACT exp
  PSUM->SBUF fp16, one matmul per pair against ctxa -> [128,130]
  (col 64/129 = row-sums via the ones columns), DVE reciprocal +
  multiply -> fp16 output in natural layout.
- Batch b's q-pass is interleaved with batch b+1's k/v-pass to keep the
  PE dense and the DMA queues evenly loaded.
"""

import numpy as np

import concourse.bass as bass
import concourse.mybir as mybir
import concourse.tile as tile
from concourse import bacc
from concourse.bass_utils import run_bass_kernel_spmd

B, N, D = 32, 16384, 64
NCORES = 8
BPC = B // NCORES  # batches per core
LOAD = 8192  # rows per DMA (1 MB fp16)
LT = LOAD // 128  # row-tile slots per load (64)
NBLK = N // LOAD  # load blocks per batch (2)
F32 = mybir.dt.float32
F16 = mybir.dt.float16
EXP = mybir.ActivationFunctionType.Exp


def build_bass():
    nc = bacc.Bacc("TRN2", target_bir_lowering=False, debug=False)
    q = nc.dram_tensor("q", [BPC, N, D], F16, kind="ExternalInput").ap()
    k = nc.dram_tensor("k", [BPC, N, D], F16, kind="ExternalInput").ap()
    v = nc.dram_tensor("v", [BPC, N, D], F16, kind="ExternalInput").ap()
    o = nc.dram_tensor("o", [BPC, N, D], F16, kind="ExternalOutput").ap()

    def blk(t, b, n0):
        return t[b, n0 : n0 + LOAD, :].rearrange("(p t) d -> p t d", p=128)

    with tile.TileContext(nc) as tc:
        with (
            tc.tile_pool(name="consts", bufs=1) as consts,
            tc.tile_pool(name="io", bufs=2) as io,
            tc.tile_pool(name="work", bufs=3) as work,
            tc.tile_pool(name="ctxp", bufs=2) as ctxp,
            tc.tile_pool(name="ps_t", bufs=2, space="PSUM") as ps_t,
            tc.tile_pool(name="ps_o", bufs=4, space="PSUM") as ps_o,
            tc.tile_pool(name="ps_c", bufs=2, space="PSUM") as ps_c,
        ):
            from concourse.masks import make_identity

            ident = consts.tile([128, 128], F16)
            make_identity(nc, ident)

            ctx_ps = {}

            def emit_kv_block(b, i):
                n0 = i * LOAD
                k_sb = io.tile([128, LT, 64], F16, tag="k_sb", bufs=4)
                v_sb = io.tile([128, LT, 64], F16, tag="v_sb", bufs=4)
                nc.sync.dma_start(out=k_sb, in_=blk(k, b, n0))
                nc.scalar.dma_start(out=v_sb, in_=blk(v, b, n0))
                ek = work.tile([128, LT, 64], F16, tag="ek")
                nc.scalar.activation(ek, k_sb, EXP)
                ks = work.tile([128, LT, 1], F32, tag="ks")
                nc.vector.reduce_sum(out=ks, in_=ek, axis=mybir.AxisListType.X)
                ksr = work.tile([128, LT, 1], F32, tag="ksr")
                nc.vector.reciprocal(ksr, ks)
                ekn = work.tile([128, LT, 64], F16, tag="ekn", bufs=4)
                nc.gpsimd.tensor_mul(ekn, ek, ksr[:].to_broadcast((128, LT, 64)))
                for t in range(LT):
                    nc.tensor.matmul(
                        ctx_ps[b],
                        ekn[:, t, :],
                        v_sb[:, t, :],
                        start=(i == 0 and t == 0),
                        stop=(i == NBLK - 1 and t == LT - 1),
                    )

            def emit_ctx_epilogue(b):
                ctxa = ctxp.tile([128, 130], F16, tag="ctxa")
                nc.vector.memset(ctxa, 0.0)
                nc.vector.tensor_copy(ctxa[0:64, 0:64], ctx_ps[b])
                nc.vector.memset(ctxa[0:64, 64:65], 1.0)
                nc.scalar.dma_start(out=ctxa[64:128, 65:130], in_=ctxa[0:64, 0:65])
                return ctxa

            def load_q_block(b, i):
                q_sb = io.tile([128, LT, 64], F16, tag="q_sb", bufs=4, name="q_sb")
                nc.sync.dma_start(out=q_sb, in_=blk(q, b, i * LOAD))
                return q_sb

            def emit_q_block(b, i, ctxa, q_sb=None, split_store=False):
                n0 = i * LOAD
                if q_sb is None:
                    q_sb = load_q_block(b, i)
                out_sb = io.tile([128, LT, 64], F16, tag="out_sb", bufs=3)
                for c in range(LT // 8):  # 1024-row compute chunks
                    tp_ps = ps_t.tile([128, 4, 128], F16, tag="tp_ps")
                    for u in range(4):
                        s0 = 8 * c + 2 * u
                        nc.tensor.transpose(
                            tp_ps[:, u, :],
                            q_sb[:, s0 : s0 + 2, :].rearrange("p t d -> p (t d)"),
                            ident,
                        )
                    eqT = work.tile([128, 4, 128], F16, tag="eqT", bufs=8)
                    nc.scalar.activation(eqT, tp_ps, EXP)
                    for g in range(2):
                        o_ps = ps_o.tile([128, 2, 132], F32, tag="o_ps")
                        for s in range(2):
                            nc.tensor.matmul(
                                o_ps[:, s, 0:130],
                                eqT[:, 2 * g + s, :],
                                ctxa,
                                start=True,
                                stop=True,
                            )
                        opb = o_ps[:]
                        pdim = opb.ap[0]
                        sstep = opb.ap[1][0]  # slot stride (132)
                        cstep = opb.ap[2][0]  # col stride (1)
                        r_sb = work.tile([128, 2, 2, 1], F32, tag="r_sb")
                        rs_ap = bass.AP(
                            tensor=opb.tensor,
                            offset=opb.offset + 64 * cstep,
                            ap=[pdim, [sstep, 2], [65 * cstep, 2], [cstep, 1]],
                        )
                        nc.vector.reciprocal(r_sb, rs_ap)
                        vals_ap = bass.AP(
                            tensor=opb.tensor,
                            offset=opb.offset,
                            ap=[pdim, [sstep, 2], [65 * cstep, 2], [cstep, 64]],
                        )
                        t0 = 8 * c + 4 * g
                        out_view = out_sb[:, t0 : t0 + 4, :].rearrange(
                            "p (s t) d -> p s t d", s=2
                        )
                        nc.vector.tensor_mul(
                            out_view,
                            vals_ap,
                            r_sb[:].to_broadcast((128, 2, 2, 64)),
                        )
                    if split_store:
                        nc.scalar.dma_start(
                            out=blk(o, b, n0)[:, 8 * c : 8 * c + 8, :],
                            in_=out_sb[:, 8 * c : 8 * c + 8, :],
                        )
                if not split_store:
                    nc.scalar.dma_start(out=blk(o, b, n0), in_=out_sb)

            # software-pipelined schedule: q-pass(b) interleaved with kv(b+1)
            ctx_ps[0] = ps_c.tile([64, 64], F32, tag="ctx_ps", name="ctx_ps")
            q_pre = [load_q_block(0, 0), load_q_block(0, 1)]
            for i in range(NBLK):
                emit_kv_block(0, i)
            ctxa = emit_ctx_epilogue(0)
            for b in range(BPC):
                if b + 1 < BPC:
                    ctx_ps[b + 1] = ps_c.tile([64, 64], F32, tag="ctx_ps", name="ctx_ps")
                nxt = None
                for i in range(NBLK):
                    # kv(b+1) first so its ctx completes before q(b) drains;
                    # epilogue right after the last kv block
                    if b + 1 < BPC:
                        emit_kv_block(b + 1, i)
                        if i == NBLK - 1:
                            nxt = emit_ctx_epilogue(b + 1)
                    last = b == BPC - 1 and i == NBLK - 1
                    emit_q_block(
                        b, i, ctxa,
                        q_sb=q_pre.pop(0) if (b == 0 and q_pre) else None,
                        split_store=last,
                    )
                if nxt is not None:
                    ctxa = nxt

    nc.compile()
    return nc


_NC_CACHE = None


def kernel(q: np.ndarray, k: np.ndarray, v: np.ndarray) -> np.ndarray:
    global _NC_CACHE
    if _NC_CACHE is None:
        _NC_CACHE = build_bass()
    nc = _NC_CACHE
    q = np.ascontiguousarray(np.asarray(q), dtype=np.float16)
    k = np.ascontiguousarray(np.asarray(k), dtype=np.float16)
    v = np.ascontiguousarray(np.asarray(v), dtype=np.float16)
    in_maps = [
        {
            "q": q[i * BPC : (i + 1) * BPC],
            "k": k[i * BPC : (i + 1) * BPC],
            "v": v[i * BPC : (i + 1) * BPC],
        }
        for i in range(NCORES)
    ]
    res = run_bass_kernel_spmd(nc, in_maps, core_ids=list(range(NCORES)))
    return np.concatenate(
        [res.results[i]["o"] for i in range(NCORES)], axis=0
    ).astype(np.float32)
